# revision 1
# baseline (speedup 1.0000x reference)
"""Kabsch loss kernel for Trainium2 (8 NeuronCores, data-parallel over batch).

Reference:
    x_c = x - mean_n(x); y_c = y - mean_n(y)
    C = x_c^T y_c  (3x3 per batch);  U,S,Vh = svd(C);  R = U Vh
    loss = mean(|x_c R - y_c|^2)
Since R is orthogonal and tr(R^T C) = tr(S):
    loss = (1/(B*N*3)) * sum_b [ |x_c|_F^2 + |y_c|_F^2 - 2*sum(svdvals(C_b)) ]
The device computes raw per-batch stats (G = x^T y, sum_x, sum_y, ssq_x, ssq_y);
the host centers them and does the 8192 tiny 3x3 SVDs in float64.
"""

import numpy as np

import jax
from jax.sharding import Mesh, NamedSharding, PartitionSpec

import concourse.bass as bass
import concourse.mybir as mybir
import concourse.tile as tile
from concourse import bass2jax

B, N = 8192, 1024
NCORES = 8
BPC = B // NCORES          # batches per core
P = 128                    # partitions
NTILES = BPC // P          # tiles of 128 batches per core
NCHUNK = N // P            # n-chunks per tile

FP32 = mybir.dt.float32

# stats layout per batch row: [G(9) | sum_x(3) | sum_y(3) | ssq_x | ssq_y]
NSTAT = 17


def _body(nc, x, y):
    st = nc.dram_tensor("stats", (BPC, NSTAT), FP32, kind="ExternalOutput")

    xr = x[:, :, :].rearrange("(t p) n c -> t p (n c)", p=P)
    yr = y[:, :, :].rearrange("(t p) n c -> t p (n c)", p=P)

    with tile.TileContext(nc) as tc:
        with (
            tc.tile_pool(name="data", bufs=3) as dpool,
            tc.tile_pool(name="scr", bufs=2) as spool,
            tc.tile_pool(name="stats", bufs=3) as stpool,
        ):
            for t in range(NTILES):
                xt = dpool.tile([P, N * 3], FP32, tag="xt")
                yt = dpool.tile([P, N * 3], FP32, tag="yt")
                nc.sync.dma_start(out=xt[:, :], in_=xr[t])
                nc.sync.dma_start(out=yt[:, :], in_=yr[t])

                xv = xt[:, :].rearrange("p (n c) -> p n c", c=3)
                yv = yt[:, :].rearrange("p (n c) -> p n c", c=3)

                sdve = stpool.tile([P, 15], FP32, tag="sdve")
                sact = stpool.tile([P, 2], FP32, tag="sact")

                # G_ij = sum_n x_i[n] * y_j[n]: 9 products then one big reduce
                prod = spool.tile([P, 9 * N], FP32, tag="prod")
                for i in range(3):
                    for j in range(3):
                        k = 3 * i + j
                        nc.vector.tensor_mul(
                            out=prod[:, k * N : (k + 1) * N],
                            in0=xv[:, :, i],
                            in1=yv[:, :, j],
                        )
                # reduce G blocks 0..6 on DVE; blocks 7,8 on ScalarE (accum)
                nc.vector.tensor_reduce(
                    out=sdve[:, 0:7],
                    in_=prod[:, 0 : 7 * N].rearrange("p (k n) -> p k n", n=N),
                    axis=mybir.AxisListType.X,
                    op=mybir.AluOpType.add,
                )
                for k in (7, 8):
                    scrg = spool.tile([P, N], FP32, tag="gred_scr")
                    nc.scalar.activation(
                        out=scrg[:, :],
                        in_=prod[:, k * N : (k + 1) * N],
                        func=mybir.ActivationFunctionType.Identity,
                        accum_out=sdve[:, k : k + 1],
                    )

                # per-coordinate sums over n on ScalarE (Identity + accumulate)
                for i in range(3):
                    scrs = spool.tile([P, N], FP32, tag="sum_scr")
                    nc.scalar.activation(
                        out=scrs[:, :],
                        in_=xv[:, :, i],
                        func=mybir.ActivationFunctionType.Identity,
                        accum_out=sdve[:, 9 + i : 10 + i],
                    )
                for i in range(3):
                    scrs = spool.tile([P, N], FP32, tag="sum_scr")
                    nc.scalar.activation(
                        out=scrs[:, :],
                        in_=yv[:, :, i],
                        func=mybir.ActivationFunctionType.Identity,
                        accum_out=sdve[:, 12 + i : 13 + i],
                    )

                # ssq via ScalarE Square + accumulate
                scrx = spool.tile([P, N * 3], FP32, tag="act_scr")
                scry = spool.tile([P, N * 3], FP32, tag="act_scr")
                nc.scalar.activation(
                    out=scrx[:, :],
                    in_=xt[:, :],
                    func=mybir.ActivationFunctionType.Square,
                    accum_out=sact[:, 0:1],
                )
                nc.scalar.activation(
                    out=scry[:, :],
                    in_=yt[:, :],
                    func=mybir.ActivationFunctionType.Square,
                    accum_out=sact[:, 1:2],
                )

                nc.sync.dma_start(out=st[t * P : (t + 1) * P, 0:15], in_=sdve[:, :])
                nc.sync.dma_start(out=st[t * P : (t + 1) * P, 15:17], in_=sact[:, :])
    return st


def _body_v2(nc, x, y):
    """PE-based stats: per 128-batch tile, strided PE transposes build
    per-coordinate planes [n, b]; a plane tile [x0,x1,x2,1 | y0,y1,y2,1]
    feeds block-diagonal Gram matmuls (32 batches x 4 cols per 128x128
    matmul, PSUM-accumulated over 8 n-chunks); mask + one selector matmul
    compacts the per-batch 4x4 Gram blocks. ssq via ScalarE Square-accum.

    gram[t, c, g*128 + bb*4 + j] = per-batch 4x4 block [xc,1]^T [yj,1]:
      [c<3, j<3] = G_cj, [c<3, 3] = sum_x_c, [3, j<3] = sum_y_j, [3,3] = N.
    """
    gram = nc.dram_tensor("gram", (NTILES, 4, 512), FP32, kind="ExternalOutput")
    sq = nc.dram_tensor("sq", (BPC, 2), FP32, kind="ExternalOutput")

    xr = x[:, :, :].rearrange("(t p) n c -> t p (n c)", p=P)
    yr = y[:, :, :].rearrange("(t p) n c -> t p (n c)", p=P)

    with tile.TileContext(nc) as tc:
        with (
            tc.tile_pool(name="const", bufs=1) as cpool,
            tc.tile_pool(name="data", bufs=3) as dpool,
            tc.tile_pool(name="planes", bufs=3) as tpool,
            tc.tile_pool(name="mstage", bufs=2) as mpool,
            tc.tile_pool(name="scr", bufs=2) as spool,
            tc.tile_pool(name="sout", bufs=3) as opool,
            tc.tile_pool(name="pt", bufs=2, space="PSUM") as ptpool,
            tc.tile_pool(name="pg", bufs=2, space="PSUM") as pgpool,
            tc.tile_pool(name="pc", bufs=2, space="PSUM") as pcpool,
        ):
            # --- constants ---
            ones = cpool.tile([P, P], FP32, tag="ones")
            nc.vector.memset(ones[:, :], 1.0)
            ident = cpool.tile([P, P], FP32, tag="ident")
            nc.gpsimd.affine_select(
                out=ident[:, :], in_=ones[:, :], pattern=[[-1, P]],
                compare_op=mybir.AluOpType.is_equal, fill=0.0,
                base=0, channel_multiplier=1,
            )
            # mask[p, (bb,j)] = 1 iff p//4 == bb  (block-diag of 4x4 blocks)
            mask0 = cpool.tile([P, P], FP32, tag="mask0")
            mask = cpool.tile([P, P], FP32, tag="mask")
            nc.gpsimd.affine_select(
                out=mask0[:, :], in_=ones[:, :], pattern=[[-4, 32], [0, 4]],
                compare_op=mybir.AluOpType.is_ge, fill=0.0,
                base=0, channel_multiplier=1,
            )
            nc.gpsimd.affine_select(
                out=mask[:, :], in_=mask0[:, :], pattern=[[-4, 32], [0, 4]],
                compare_op=mybir.AluOpType.is_le, fill=0.0,
                base=-3, channel_multiplier=1,
            )
            # sel[p, c] = 1 iff p % 4 == c   (= sum_b ident[p, b*4+c])
            sel = cpool.tile([P, 4], FP32, tag="sel")
            nc.vector.tensor_reduce(
                out=sel[:, :],
                in_=ident[:, :].rearrange("p (b c) -> p c b", c=4),
                axis=mybir.AxisListType.X,
                op=mybir.AluOpType.add,
            )

            for t in range(NTILES):
                xt = dpool.tile([P, N * 3], FP32, tag="xt")
                yt = dpool.tile([P, N * 3], FP32, tag="yt")
                nc.sync.dma_start(out=xt[:, :], in_=xr[t])
                nc.sync.dma_start(out=yt[:, :], in_=yr[t])
                xv = xt[:, :].rearrange("p (n c) -> p n c", c=3)
                yv = yt[:, :].rearrange("p (n c) -> p n c", c=3)

                # ssq via ScalarE Square + accumulate
                sact = opool.tile([P, 2], FP32, tag="sact")
                scrx = spool.tile([P, N * 3], FP32, tag="act_scr")
                scry = spool.tile([P, N * 3], FP32, tag="act_scr")
                nc.scalar.activation(
                    out=scrx[:, :], in_=xt[:, :],
                    func=mybir.ActivationFunctionType.Square,
                    accum_out=sact[:, 0:1],
                )
                nc.scalar.activation(
                    out=scry[:, :], in_=yt[:, :],
                    func=mybir.ActivationFunctionType.Square,
                    accum_out=sact[:, 1:2],
                )

                # Gram accumulation PSUM: 4 groups of 32 batches side by side
                pg = pgpool.tile([P, 512], FP32, tag="pg")
                for q in range(NCHUNK):
                    ptx = ptpool.tile([P, 3 * P], FP32, tag="ptx")
                    pty = ptpool.tile([P, 3 * P], FP32, tag="pty")
                    for c in range(3):
                        nc.tensor.transpose(
                            out=ptx[:, c * P : (c + 1) * P],
                            in_=xv[:, q * P : (q + 1) * P, c],
                            identity=ident[:, :],
                        )
                        nc.tensor.transpose(
                            out=pty[:, c * P : (c + 1) * P],
                            in_=yv[:, q * P : (q + 1) * P, c],
                            identity=ident[:, :],
                        )
                    tq = tpool.tile([P, 8 * P], FP32, tag="tq")
                    nc.vector.tensor_copy(tq[:, 0 : 3 * P], ptx[:, :])
                    nc.scalar.copy(out=tq[:, 4 * P : 7 * P], in_=pty[:, :])
                    nc.gpsimd.memset(tq[:, 3 * P : 4 * P], 1.0)
                    nc.gpsimd.memset(tq[:, 7 * P : 8 * P], 1.0)
                    tqv = tq[:, :].rearrange("p (pl b) -> p b pl", pl=8)
                    for g in range(4):
                        nc.tensor.matmul(
                            out=pg[:, g * P : (g + 1) * P],
                            lhsT=tqv[:, g * 32 : (g + 1) * 32, 0:4],
                            rhs=tqv[:, g * 32 : (g + 1) * 32, 4:8],
                            start=(q == 0),
                            stop=(q == NCHUNK - 1),
                        )

                # extract per-batch diagonal 4x4 blocks
                mstage = mpool.tile([P, 512], FP32, tag="mstage")
                for g in range(4):
                    nc.vector.tensor_mul(
                        out=mstage[:, g * P : (g + 1) * P],
                        in0=pg[:, g * P : (g + 1) * P],
                        in1=mask[:, :],
                    )
                pc = pcpool.tile([4, 512], FP32, tag="pc")
                nc.tensor.matmul(
                    out=pc[:, :], lhsT=sel[:, :], rhs=mstage[:, :],
                    start=True, stop=True,
                )
                gstage = opool.tile([4, 512], FP32, tag="gstage")
                nc.vector.tensor_copy(gstage[:, :], pc[:, :])
                nc.sync.dma_start(out=gram[t], in_=gstage[:, :])
                nc.sync.dma_start(out=sq[t * P : (t + 1) * P, :], in_=sact[:, :])
    return gram, sq


VERSION = 1
_CACHE = {}


def _get_runner():
    if "runner" not in _CACHE:
        if VERSION == 1:
            jitted = bass2jax.bass_jit(_body)
            out_specs = PartitionSpec("core")
        else:
            jitted = bass2jax.bass_jit(_body_v2)
            out_specs = (PartitionSpec("core"), PartitionSpec("core"))
        devices = jax.devices()[:NCORES]
        mesh = Mesh(np.asarray(devices), ("core",))
        f = bass2jax.bass_shard_map(
            jitted,
            mesh=mesh,
            in_specs=(PartitionSpec("core"), PartitionSpec("core")),
            out_specs=out_specs,
        )
        _CACHE["runner"] = (f, mesh)
    return _CACHE["runner"]


def _host_finish(stats: np.ndarray) -> np.ndarray:
    s = stats.astype(np.float64)
    nb = s.shape[0]
    G = s[:, 0:9].reshape(nb, 3, 3)
    sx = s[:, 9:12]
    sy = s[:, 12:15]
    ssx = s[:, 15]
    ssy = s[:, 16]
    C = G - sx[:, :, None] * sy[:, None, :] / N
    nuc = np.linalg.svd(C, compute_uv=False).sum(1)
    ssxc = ssx - (sx**2).sum(1) / N
    ssyc = ssy - (sy**2).sum(1) / N
    loss = (ssxc + ssyc - 2.0 * nuc).sum() / (nb * N * 3)
    return np.asarray(loss, dtype=np.float32)


def _host_finish_v2(gram: np.ndarray, sq: np.ndarray) -> np.ndarray:
    # gram: (NCORES*NTILES, 4, 512), f-dim = (g:4, bb:32, j:4)
    gr = gram.astype(np.float64).reshape(-1, 4, 4, 32, 4)  # [tile, c, g, bb, j]
    gr = gr.transpose(0, 2, 3, 1, 4).reshape(B, 4, 4)       # [b, c, j]
    G = gr[:, 0:3, 0:3]
    sx = gr[:, 0:3, 3]
    sy = gr[:, 3, 0:3]
    s = sq.astype(np.float64)
    ssx = s[:, 0]
    ssy = s[:, 1]
    C = G - sx[:, :, None] * sy[:, None, :] / N
    nuc = np.linalg.svd(C, compute_uv=False).sum(1)
    ssxc = ssx - (sx**2).sum(1) / N
    ssyc = ssy - (sy**2).sum(1) / N
    loss = (ssxc + ssyc - 2.0 * nuc).sum() / (B * N * 3)
    return np.asarray(loss, dtype=np.float32)


def kernel(x, y):
    f, _ = _get_runner()
    x = np.ascontiguousarray(np.asarray(x, dtype=np.float32))
    y = np.ascontiguousarray(np.asarray(y, dtype=np.float32))
    out = jax.block_until_ready(f(x, y))
    if VERSION == 1:
        return _host_finish(np.asarray(out))
    gram, sq = out
    return _host_finish_v2(np.asarray(gram), np.asarray(sq))


def bench(x, y, iters=10):
    import time

    f, mesh = _get_runner()
    sh = NamedSharding(mesh, PartitionSpec("core"))
    xd = jax.device_put(np.asarray(x, dtype=np.float32), sh)
    yd = jax.device_put(np.asarray(y, dtype=np.float32), sh)
    jax.block_until_ready(f(xd, yd))  # warm up / compile
    times = []
    for _ in range(iters):
        t0 = time.perf_counter()
        jax.block_until_ready(f(xd, yd))
        times.append(time.perf_counter() - t0)
    return times



# revision 2
# speedup vs baseline: 766.5283x; 766.5283x over previous
"""Kabsch loss kernel for Trainium2 (8 NeuronCores, data-parallel over batch).

Reference:
    x_c = x - mean_n(x); y_c = y - mean_n(y)
    C = x_c^T y_c  (3x3 per batch);  U,S,Vh = svd(C);  R = U Vh
    loss = mean(|x_c R - y_c|^2)
Since R is orthogonal and tr(R^T C) = tr(S):
    loss = (1/(B*N*3)) * sum_b [ |x_c|_F^2 + |y_c|_F^2 - 2*sum(svdvals(C_b)) ]
The device computes raw per-batch stats (G = x^T y, sum_x, sum_y, ssq_x, ssq_y);
the host centers them and does the 8192 tiny 3x3 SVDs in float64.

v3 device kernel (all fp32), per 128-batch tile:
  - de-interleave (b, (n c)) -> 6 coordinate planes, with the per-coordinate
    sums accumulated for free: DVE tensor_scalar(+accum) for 2 planes,
    ACT activation(Identity, accum_out) for 4 planes
  - 9 fused product+reduce ops (DVE scalar_tensor_tensor with accum_out)
    on the unit-stride planes -> G entries
  - 2 fused square+reduce ops (ACT activation(Square, accum_out)) on the
    raw interleaved tiles -> ssq_x, ssq_y
This keeps DVE ~= ACT ~= 14us/tile and overlaps with the ~9.5us/tile DMA.
"""

import numpy as np

import jax
from jax.sharding import Mesh, NamedSharding, PartitionSpec

import concourse.bass as bass
import concourse.mybir as mybir
import concourse.tile as tile
from concourse import bass2jax

B, N = 8192, 1024
NCORES = 8
BPC = B // NCORES          # batches per core
P = 128                    # partitions
NTILES = BPC // P          # tiles of 128 batches per core

FP32 = mybir.dt.float32

# stats layout per batch row: [G(9) | sum_x(3) | sum_y(3) | ssq_x | ssq_y]
NSTAT = 17


def _body(nc, x, y):
    st = nc.dram_tensor("stats", (BPC, NSTAT), FP32, kind="ExternalOutput")

    xr = x[:, :, :].rearrange("(t p) n c -> t p (n c)", p=P)
    yr = y[:, :, :].rearrange("(t p) n c -> t p (n c)", p=P)

    MUL = mybir.AluOpType.mult
    ADD = mybir.AluOpType.add

    with tile.TileContext(nc) as tc:
        with (
            tc.tile_pool(name="data", bufs=3) as dpool,
            tc.tile_pool(name="planes", bufs=2) as ppool,
            tc.tile_pool(name="scr", bufs=2) as spool,
            tc.tile_pool(name="stats", bufs=3) as stpool,
        ):
            for t in range(NTILES):
                xt = dpool.tile([P, N * 3], FP32, tag="xt")
                yt = dpool.tile([P, N * 3], FP32, tag="yt")
                nc.sync.dma_start(out=xt[:, :], in_=xr[t])
                nc.sync.dma_start(out=yt[:, :], in_=yr[t])

                xv = xt[:, :].rearrange("p (n c) -> p n c", c=3)
                yv = yt[:, :].rearrange("p (n c) -> p n c", c=3)

                sdve = stpool.tile([P, NSTAT], FP32, tag="sdve")

                planes = ppool.tile([P, 6 * N], FP32, tag="planes")
                pv = planes[:, :].rearrange("p (c n) -> p c n", c=6)

                # --- de-interleave + per-coordinate sums (fused) ---
                # x0, x1 on DVE via tensor_scalar(+accum)
                for i in range(2):
                    nc.vector.tensor_scalar(
                        out=pv[:, i, :], in0=xv[:, :, i],
                        scalar1=1.0, scalar2=0.0, op0=MUL, op1=ADD,
                        accum_out=sdve[:, 9 + i : 10 + i],
                    )
                # x2, y0, y1, y2 on ACT via Identity(+accum)
                nc.scalar.activation(
                    out=pv[:, 2, :], in_=xv[:, :, 2],
                    func=mybir.ActivationFunctionType.Identity,
                    accum_out=sdve[:, 11:12],
                )
                for j in range(3):
                    nc.scalar.activation(
                        out=pv[:, 3 + j, :], in_=yv[:, :, j],
                        func=mybir.ActivationFunctionType.Identity,
                        accum_out=sdve[:, 12 + j : 13 + j],
                    )

                # --- ssq via ACT Square(+accum) on the raw interleaved tiles ---
                scrx = spool.tile([P, N * 3], FP32, tag="act_scr")
                nc.scalar.activation(
                    out=scrx[:, :], in_=xt[:, :],
                    func=mybir.ActivationFunctionType.Square,
                    accum_out=sdve[:, 15:16],
                )
                scry = spool.tile([P, N * 3], FP32, tag="act_scr2")
                nc.scalar.activation(
                    out=scry[:, :], in_=yt[:, :],
                    func=mybir.ActivationFunctionType.Square,
                    accum_out=sdve[:, 16:17],
                )

                # --- G_ij = sum_n x_i y_j : fused product+reduce on DVE ---
                prod = spool.tile([P, N], FP32, tag="prod")
                for i in range(3):
                    for j in range(3):
                        k = 3 * i + j
                        nc.vector.scalar_tensor_tensor(
                            out=prod[:, :], in0=pv[:, i, :], scalar=1.0,
                            in1=pv[:, 3 + j, :], op0=MUL, op1=MUL,
                            accum_out=sdve[:, k : k + 1],
                        )

                nc.sync.dma_start(out=st[t * P : (t + 1) * P, :], in_=sdve[:, :])
    return st


VERSION = 3
_CACHE = {}


def _get_runner():
    if "runner" not in _CACHE:
        jitted = bass2jax.bass_jit(_body)
        out_specs = PartitionSpec("core")
        devices = jax.devices()[:NCORES]
        mesh = Mesh(np.asarray(devices), ("core",))
        f = bass2jax.bass_shard_map(
            jitted,
            mesh=mesh,
            in_specs=(PartitionSpec("core"), PartitionSpec("core")),
            out_specs=out_specs,
        )
        _CACHE["runner"] = (f, mesh)
    return _CACHE["runner"]


def _host_finish(stats: np.ndarray) -> np.ndarray:
    s = stats.astype(np.float64)
    nb = s.shape[0]
    G = s[:, 0:9].reshape(nb, 3, 3)
    sx = s[:, 9:12]
    sy = s[:, 12:15]
    ssx = s[:, 15]
    ssy = s[:, 16]
    C = G - sx[:, :, None] * sy[:, None, :] / N
    nuc = np.linalg.svd(C, compute_uv=False).sum(1)
    ssxc = ssx - (sx**2).sum(1) / N
    ssyc = ssy - (sy**2).sum(1) / N
    loss = (ssxc + ssyc - 2.0 * nuc).sum() / (nb * N * 3)
    return np.asarray(loss, dtype=np.float32)


def kernel(x, y):
    f, _ = _get_runner()
    x = np.ascontiguousarray(np.asarray(x, dtype=np.float32))
    y = np.ascontiguousarray(np.asarray(y, dtype=np.float32))
    out = jax.block_until_ready(f(x, y))
    return _host_finish(np.asarray(out))


def bench(x, y, iters=10):
    import time

    f, mesh = _get_runner()
    sh = NamedSharding(mesh, PartitionSpec("core"))
    xd = jax.device_put(np.asarray(x, dtype=np.float32), sh)
    yd = jax.device_put(np.asarray(y, dtype=np.float32), sh)
    jax.block_until_ready(f(xd, yd))  # warm up / compile
    times = []
    for _ in range(iters):
        t0 = time.perf_counter()
        jax.block_until_ready(f(xd, yd))
        times.append(time.perf_counter() - t0)
    return times
